# revision 10
# baseline (speedup 1.0000x reference)
"""AsymmetricSVD segment-reduce kernel for 8 TRN2 NeuronCores (v2).

Strategy (data-parallel over segments, fp8 + DoubleRow):
  - Core m owns segments [512m, 512(m+1)) and their contiguous implicit
    entries (segment_ids is sorted).
  - Host precomputes per-entry scalar a_e = r_e - MU - bu[user[seg_e]] and a
    fused fp8 table XY = 128*[X | Y - bi*X] (so w*X + Y == a*X + Y'; the
    2^7 scale keeps fp8e4 out of subnormals and is folded back in Qn2).
  - Entries are bucketed by item range (4 buckets of 25000 rows so gather
    indices fit int16) and, within a bucket, grouped by 32-segment
    superblock.  Each (bucket, superblock) run is padded to a multiple of
    256 entries (cross-core max) so every 256-entry PAIR of gather groups
    lies inside one 32-segment window -> a single PSUM strip [p0:p0+32)
    with p0 in {0,32,64,96}.
  - Device gathers 256B fp8 rows per entry via gpsimd.dma_gather (SWDGE).
  - The one-hot/coefficient lhsT tiles are PRE-BUILT ON HOST in fp8
    ([128, pairs, 4, 32]: Sp k0, Sp k1, S k0, S k1) and streamed in via
    HWDGE - no on-device one-hot construction at all.
  - Per pair, two fp8 DoubleRow matmuls (256 entries each, 0.5 cyc/row):
        PSUM[strip, 0:128]   += sum_e a_e * X_e      (lhsT = Sp pair)
        PSUM[strip, 128:256] += sum_e (Y')_e         (lhsT = S pair)
  - Epilogue: rui[seg] = bui[seg] + reduce_add(PSUM[seg, 0:256] * Qn2[seg])
    with Qn2 = [Qn | Qn], Qn = Q[item]*norm/128 precomputed on host.
"""

import numpy as np
import ml_dtypes

MU = 3.5
B = 4096
F = 128
NUM_ITEMS = 100000
N_CORES = 8
SEGS_PER_CORE = B // N_CORES            # 512
N_BUCKETS = 4
BUCKET_ROWS = (NUM_ITEMS + N_BUCKETS - 1) // N_BUCKETS   # 25000 < 32768 (int16)
SB = 32                                  # segments per superblock
NSB = SEGS_PER_CORE // SB                # 16 superblocks per core
PAIR = 256                               # entries per DoubleRow matmul pair
CHUNK = 3584                             # entries per dma_gather call (14 pairs)
FP8 = ml_dtypes.float8_e4m3
XSCALE = 128.0                           # fp8 range scale for X/Y' rows


def _host_prep(bu, bi, Q, X, Y, user, item, imp_items, imp_ratings, segment_ids):
    """All index/scalar preprocessing. Returns per-core device arrays and
    uniform cross-core metadata for codegen."""
    a_full = imp_ratings.astype(np.float32) - MU - bu[user[segment_ids], 0]
    Yp = Y - bi * X                                    # [NUM_ITEMS, F]
    XYs = np.concatenate([X * XSCALE, Yp * XSCALE], axis=1).astype(FP8)

    counts = np.bincount(segment_ids, minlength=B).astype(np.float32)
    norm = np.where(counts > 0, counts, 1.0) ** -0.5
    bui = (MU + bu[user, 0] + bi[item, 0]).astype(np.float32)          # [B]
    Qh = (Q[item] * (norm / XSCALE)[:, None]).astype(np.float32)       # [B, F]
    Qn2 = np.concatenate([Qh, Qh], axis=1)                             # [B, 256]

    # --- shard entries by segment block; sort by (bucket, superblock) ---
    bounds = np.searchsorted(segment_ids, np.arange(0, B + 1, SEGS_PER_CORE))
    percore = []
    cnt = np.zeros((N_CORES, N_BUCKETS, NSB), np.int64)
    for m in range(N_CORES):
        lo, hi = bounds[m], bounds[m + 1]
        it = imp_items[lo:hi]
        sl = (segment_ids[lo:hi] - m * SEGS_PER_CORE).astype(np.int64)
        av = a_full[lo:hi]
        bk = it // BUCKET_ROWS
        key = bk * NSB + sl // SB
        order = np.argsort(key, kind="stable")
        it, sl, av, key = it[order], sl[order], av[order], key[order]
        cnt[m] = np.bincount(key, minlength=N_BUCKETS * NSB).reshape(
            N_BUCKETS, NSB)
        percore.append((it, sl, av, key))

    # uniform per-(bucket, superblock) capacity: cross-core max, pair-aligned
    cap = ((cnt.max(axis=0) + PAIR - 1) // PAIR) * PAIR       # [4, 16]
    offs_flat = np.concatenate([[0], np.cumsum(cap.ravel())])  # [65]
    E_pad = int(offs_flat[-1])
    G2 = E_pad // PAIR                                         # pairs

    # pair -> (psum tile, col base) metadata (uniform across cores).
    # DoubleRow matmuls require PSUM base partition 0, so each superblock
    # gets its own rows-0:32 region: tile t = bank t holds superblock 2t at
    # cols 0:256 and superblock 2t+1 at cols 256:512.
    pair_meta = []
    for b in range(N_BUCKETS):
        for sb in range(NSB):
            npairs = int(cap[b, sb]) // PAIR
            bank, cb = sb // 2, 256 * (sb % 2)
            pair_meta.extend([(bank, cb)] * npairs)
    assert len(pair_meta) == G2

    # chunk list per bucket: (start_entry, n_entries, bucket), pair-aligned
    chunks = []
    bucket_bounds = []
    for b in range(N_BUCKETS):
        s = int(offs_flat[b * NSB])
        nb = int(cap[b].sum())
        bucket_bounds.append((s, nb))
        npair_b = nb // PAIR
        ncalls = max(1, (nb + CHUNK - 1) // CHUNK)
        base_p, extra = divmod(npair_b, ncalls)
        for c in range(ncalls):
            g = base_p + (1 if c < extra else 0)
            if g == 0:
                continue
            n = g * PAIR
            chunks.append((s, n, b))
            s += n

    meta = dict(E_pad=E_pad, G2=G2, chunks=chunks, pair_meta=pair_meta,
                bucket_bounds=bucket_bounds)

    # --- per-core device arrays ---
    def wrap16(x):   # entry e -> [e%16, e//16], replicated to 128 partitions
        w = x.reshape(-1, 16).T
        return np.ascontiguousarray(np.tile(w, (8, 1)))

    in_maps = []
    for m in range(N_CORES):
        it, sl, av, key = percore[m]
        ne = it.shape[0]
        # slot of each (sorted) entry inside the padded uniform stream
        gstart = np.concatenate([[0], np.cumsum(cnt[m].ravel())])
        rank = np.arange(ne) - gstart[key]
        slot = offs_flat[key] + rank

        lidx = np.zeros(E_pad, np.int16)
        lidx[slot] = (it - (it // BUCKET_ROWS) * BUCKET_ROWS).astype(np.int16)

        # lhsT coefficients: [128, G2, 4, 32] = (Sp k0, Sp k1, S k0, S k1)
        p = slot // PAIR
        k = (slot % PAIR) // 128
        part = slot % 128
        col = sl - SB * (key % NSB)
        LTf = np.zeros((128, G2, 4, SB), np.float32)
        LTf[part, p, k, col] = av
        LTf[part, p, 2 + k, col] = 1.0

        in_maps.append({
            "xy": XYs,
            "idx16": wrap16(lidx),
            "lt": LTf.astype(FP8),
            "qn2": np.ascontiguousarray(Qn2[m * SEGS_PER_CORE:(m + 1) * SEGS_PER_CORE]),
            "bui": np.ascontiguousarray(bui[m * SEGS_PER_CORE:(m + 1) * SEGS_PER_CORE]),
        })
    return in_maps, meta


def _build_graph(meta):
    from concourse import bacc, mybir
    from concourse.tile import TileContext

    E_pad, G2 = meta["E_pad"], meta["G2"]
    chunks, pair_meta = meta["chunks"], meta["pair_meta"]
    bucket_bounds = meta["bucket_bounds"]

    nc = bacc.Bacc("TRN2", target_bir_lowering=False, debug=False,
                   num_devices=N_CORES, num_swdge_queues=4)
    fp8, f32, i16 = mybir.dt.float8e4, mybir.dt.float32, mybir.dt.int16
    DR = mybir.MatmulPerfMode.DoubleRow

    xy_d = nc.declare_dram_parameter("xy", [NUM_ITEMS, 256], fp8, isOutput=False)
    idx_d = nc.declare_dram_parameter("idx16", [128, E_pad // 16], i16, isOutput=False)
    lt_d = nc.declare_dram_parameter("lt", [128, G2, 4, SB], fp8, isOutput=False)
    qn_d = nc.declare_dram_parameter("qn2", [SEGS_PER_CORE, 256], f32, isOutput=False)
    bui_d = nc.declare_dram_parameter("bui", [SEGS_PER_CORE], f32, isOutput=False)
    out_d = nc.declare_dram_parameter("out", [SEGS_PER_CORE], f32, isOutput=True)

    # 8 PSUM bank tiles; superblock sb lives at rows 0:32, cols
    # 256*(sb%2) : 256*(sb%2)+256 of bank sb//2.
    n_tiles = NSB // 2  # 8

    with TileContext(nc) as tc:
        with (
            tc.tile_pool(name="const", bufs=1) as cpool,
            tc.tile_pool(name="xy", bufs=6) as xypool,
            tc.tile_pool(name="epi", bufs=2) as epool,
            tc.tile_pool(name="psum", bufs=1, space="PSUM") as ppool,
        ):
            # per-bucket idx + lhsT tiles, interleaved so bucket 0 is ready
            # first; idx on the sync HWDGE queue (gates gathers), lhsT on
            # the scalar queue.
            idx_tiles, lt_tiles, lt_poff = [], [], []
            for b in range(N_BUCKETS):
                boff, bn = bucket_bounds[b]
                if bn == 0:
                    idx_tiles.append(None)
                    lt_tiles.append(None)
                    lt_poff.append(0)
                    continue
                t = cpool.tile([128, bn // 16], i16, tag=f"idx{b}")
                nc.sync.dma_start(
                    out=t[:], in_=idx_d[:, boff // 16:(boff + bn) // 16])
                idx_tiles.append(t)
                p0b, npb = boff // PAIR, bn // PAIR
                lt = cpool.tile([128, npb, 4, SB], fp8, tag=f"lt{b}")
                nc.scalar.dma_start(out=lt[:], in_=lt_d[:, p0b:p0b + npb, :, :])
                lt_tiles.append(lt)
                lt_poff.append(p0b)

            zeros_t = cpool.tile([128, 512], fp8, tag="zeros")
            nc.vector.memset(zeros_t[:], 0.0)

            psum_t = []
            for kbank in range(n_tiles):
                pt = ppool.tile([128, 512], f32, tag=f"bank{kbank}")
                psum_t.append(pt)
                nc.tensor.matmul(
                    out=pt[0:32, 0:512], lhsT=zeros_t[:, 0:32],
                    rhs=zeros_t[:, 0:512], start=True, stop=False,
                )

            for ci, (start, n, b) in enumerate(chunks):
                nG, nP = n // 128, n // PAIR
                boff = bucket_bounds[b][0]
                xyt = xypool.tile([128, nG, 256], fp8, tag="xyt")
                nc.gpsimd.dma_gather(
                    out_ap=xyt[:],
                    in_ap=xy_d[b * BUCKET_ROWS:(b + 1) * BUCKET_ROWS, :],
                    idxs_ap=idx_tiles[b][:, (start - boff) // 16:
                                         (start - boff + n) // 16],
                    num_idxs=n,
                    num_idxs_reg=n,
                    elem_size=256,
                    single_packet=False,
                    queue_num=ci % 4,
                )
                ltb = lt_tiles[b]
                for u in range(nP):
                    P = start // PAIR + u
                    Pb = P - lt_poff[b]
                    bank, cb = pair_meta[P]
                    for which in (0, 1):
                        c0, c1 = 128 * which, 128 * (which + 1)
                        nc.tensor.matmul(
                            out=psum_t[bank][0:SB, cb + c0:cb + c1],
                            lhsT=ltb[:, Pb, 2 * which:2 * which + 2, :],
                            rhs=xyt[:, 2 * u:2 * u + 2, c0:c1],
                            start=False, stop=False,
                            perf_mode=DR,
                        )

            # close accumulation groups (full-width, required before reads)
            for kbank in range(n_tiles):
                nc.tensor.matmul(
                    out=psum_t[kbank][0:32, 0:512], lhsT=zeros_t[:, 0:32],
                    rhs=zeros_t[:, 0:512], start=False, stop=True,
                )

            # epilogue: superblock sb = rows 0:32, cols 256*(sb%2)+0:256 of
            # bank sb//2; covers segments [32sb, 32sb+32)
            for sb in range(NSB):
                bank, cb = sb // 2, 256 * (sb % 2)
                s0 = SB * sb
                bui_t = epool.tile([128, 1], f32, tag="bui")
                nc.scalar.dma_start(
                    out=bui_t[0:SB, :], in_=bui_d[s0:s0 + SB])
                qn_t = epool.tile([128, 256], f32, tag="qn")
                nc.scalar.dma_start(
                    out=qn_t[0:SB, :], in_=qn_d[s0:s0 + SB, :])
                prod_t = epool.tile([128, 256], f32, tag="prod")
                nc.vector.tensor_tensor(
                    out=prod_t[0:SB, :], in0=psum_t[bank][0:SB, cb:cb + 256],
                    in1=qn_t[0:SB, :],
                    op=mybir.AluOpType.mult,
                )
                red_t = epool.tile([128, 1], f32, tag="red")
                nc.vector.tensor_reduce(
                    out=red_t[0:SB, 0:1], in_=prod_t[0:SB, :],
                    axis=mybir.AxisListType.X,
                    op=mybir.AluOpType.add,
                )
                nc.vector.tensor_add(red_t[0:SB, 0:1], red_t[0:SB, 0:1],
                                     bui_t[0:SB, 0:1])
                nc.sync.dma_start(
                    out=out_d[s0:s0 + SB], in_=red_t[0:SB, 0:1])

    nc.compile()
    return nc


def kernel(bu, bi, Q, X, Y, user, item, imp_items, imp_ratings, segment_ids,
           _sim=False):
    bu = np.asarray(bu, np.float32)
    bi = np.asarray(bi, np.float32)
    Q = np.asarray(Q, np.float32)
    X = np.asarray(X, np.float32)
    Y = np.asarray(Y, np.float32)
    user = np.asarray(user).astype(np.int64)
    item = np.asarray(item).astype(np.int64)
    imp_items = np.asarray(imp_items).astype(np.int64)
    imp_ratings = np.asarray(imp_ratings).astype(np.int64)
    segment_ids = np.asarray(segment_ids).astype(np.int64)

    in_maps, meta = _host_prep(bu, bi, Q, X, Y, user, item, imp_items,
                               imp_ratings, segment_ids)
    nc = _build_graph(meta)

    if _sim:
        from concourse import bass_interp
        sim = bass_interp.CoreSim(nc)
        sim.assign_tensors(in_maps[0])
        sim.simulate()
        out0 = np.array(sim.tensor("out"))
        return sim, out0, in_maps, meta

    from concourse.bass_utils import run_bass_kernel_spmd
    res = run_bass_kernel_spmd(nc, in_maps, core_ids=list(range(N_CORES)),
                               trace=False)
    out = np.concatenate([res.results[m]["out"] for m in range(N_CORES)])
    return out.astype(np.float32)
